# revision 50
# baseline (speedup 1.0000x reference)
"""Trainium2 Bass kernel for CascadedPathEncoder.

Reference computation (per sample b):
    h_0 = relu(W_0 @ [0_256; wp_0] + b_0)
    h_p = relu(W_p @ [h_{p-1}; wp_p] + b_p)      p = 1..31
    out[b] = concat_p h_p                         -> [8192, 8192]

Strategy: pure data parallel over 8 NeuronCores (1024 batch rows each).
Per core the hidden state is kept transposed in SBUF as two bf16
[128, 1024] chunks (partition dim = hidden index). Each step runs, per
512-column batch tile and per output chunk m, three matmuls that
accumulate in PSUM: K=4 (the wp contribution, lhsT from W[:, :, 256:260])
and two K=128 chunks (lhsT from W[:, :, :256]). Bias + relu are fused on
the Activation engine (m=0) and Vector engine (m=1), writing the new
bf16 state chunk, which is also DMA'd to DRAM as the step's output
slice. Host un-transposes / re-assembles the full [8192, 8192] f32.
"""

import numpy as np
import ml_dtypes

BF16 = ml_dtypes.bfloat16
P = 32          # scan steps
PD = 4          # point dim
H = 256         # hidden dim
B = 8192        # global batch
NCORES = 8
BS = B // NCORES  # 1024 rows per core
TN = 512        # matmul moving free dim (one PSUM bank of f32; ISA max)
NT = BS // TN   # batch tiles per core

_CACHE = {}


def _build_nc():
    from contextlib import ExitStack

    import concourse.bass as bass
    import concourse.tile as tile
    from concourse import bacc, mybir

    dt = mybir.dt
    ts = bass.ts

    nc = bacc.Bacc(
        "TRN2", target_bir_lowering=False, debug=False, num_devices=NCORES
    )
    WH_CHUNK = 8  # steps per wh DMA chunk (pipeline weight loads)
    wh = nc.dram_tensor("wh", [128, P, 2, 256], dt.bfloat16, kind="ExternalInput").ap()
    # wx is stored as zero-padded K=64 lhsT blocks so both wx and pdx can be
    # read at matmul-legal base partitions (0/64) while their DMAs stay full
    # 128-partition width: wx[4p+r, p, m, j] = W[p, 128m+j, 256+r], zero
    # elsewhere; pdx[4p+r, b] = path_data[b, 4p+r].
    wx = nc.dram_tensor("wx", [128, P, 2, 128], dt.bfloat16, kind="ExternalInput").ap()
    pdx = nc.dram_tensor("pdx", [128, BS], dt.bfloat16, kind="ExternalInput").ap()
    bias = nc.dram_tensor("bias", [128, P, 2], dt.float32, kind="ExternalInput").ap()
    out = nc.dram_tensor(
        "out", [P, 128, 2, NT, TN], dt.bfloat16, kind="ExternalOutput"
    ).ap()

    with tile.TileContext(nc) as tc, ExitStack() as ctx:
        const = ctx.enter_context(tc.tile_pool(name="const", bufs=1))
        state = ctx.enter_context(tc.tile_pool(name="state", bufs=3))
        psum = ctx.enter_context(tc.tile_pool(name="psum", bufs=2, space="PSUM"))

        # Order the input DMAs by first-use; all transfers are full width.
        wx_sb = const.tile([128, P, 2, 128], dt.bfloat16)
        pdx_sb = const.tile([128, BS], dt.bfloat16)
        b_sb = const.tile([128, P, 2], dt.float32)
        wh_sb = const.tile([128, P, 2, 256], dt.bfloat16)

        nc.sync.dma_start(out=pdx_sb[:], in_=pdx[:])
        nc.sync.dma_start(out=wx_sb[:, 0:2, :, :], in_=wx[:, 0:2, :, :])
        nc.sync.dma_start(out=b_sb[:], in_=bias[:])
        # front-load the first half of wh: steps 1..15 are consumed faster
        # than the sync HWDGE ring can stream them, and stalls here are on
        # the critical path
        for g in range(2):
            sl = slice(g * WH_CHUNK, (g + 1) * WH_CHUNK)
            nc.sync.dma_start(out=wh_sb[:, sl, :, :], in_=wh[:, sl, :, :])
        nc.sync.dma_start(out=wx_sb[:, 2:8, :, :], in_=wx[:, 2:8, :, :])
        for g in range(2, P // WH_CHUNK):
            sl = slice(g * WH_CHUNK, (g + 1) * WH_CHUNK)
            nc.sync.dma_start(out=wh_sb[:, sl, :, :], in_=wh[:, sl, :, :])
            slx = slice(8 * (g - 1), 8 * g)
            nc.sync.dma_start(out=wx_sb[:, slx, :, :], in_=wx[:, slx, :, :])
        nc.sync.dma_start(out=wx_sb[:, 24:32, :, :], in_=wx[:, 24:32, :, :])

        h_prev = None
        for p in range(P):
            ps = [
                psum.tile(
                    [128, NT, TN],
                    dt.float32,
                    tag=f"ps_m{m}",
                    name=f"ps_p{p}m{m}",
                )
                for m in range(2)
            ]
            # K=4 wp contribution opens each accumulation group. t is
            # innermost everywhere so consecutive matmuls share lhsT (one
            # weight load covers NT matmuls). k outer / m inner for the
            # K=128 chunks: the k=0 matmuls only need the previous step's
            # m=0 relu, giving the m=1 relu a longer window to complete.
            base = 64 * (p // 16)
            for m in range(2):
                for t in range(NT):
                    nc.tensor.matmul(
                        ps[m][:, t, :],
                        lhsT=wx_sb[base : base + 64, p, m, :],
                        rhs=pdx_sb[base : base + 64, ts(t, TN)],
                        start=True,
                        stop=(p == 0),
                    )
            if p > 0:
                for k in range(2):
                    for m in range(2):
                        for t in range(NT):
                            nc.tensor.matmul(
                                ps[m][:, t, :],
                                lhsT=wh_sb[:, p, k, ts(m, 128)],
                                rhs=h_prev[:, k, t, :],
                                start=False,
                                stop=(k == 1),
                            )
            hn = state.tile(
                [128, 2, NT, TN], dt.bfloat16, tag="h", name=f"h_p{p}"
            )
            for m in range(2):
                if m == 0:
                    nc.scalar.activation(
                        hn[:, m, :, :],
                        ps[m][:],
                        mybir.ActivationFunctionType.Relu,
                        bias=b_sb[:, p, m : m + 1],
                        scale=1.0,
                    )
                else:
                    nc.vector.tensor_scalar(
                        hn[:, m, :, :],
                        ps[m][:],
                        scalar1=b_sb[:, p, m : m + 1],
                        scalar2=0.0,
                        op0=mybir.AluOpType.add,
                        op1=mybir.AluOpType.max,
                    )
            if p >= P - 2:
                # tail: split by batch half so the final transfers start
                # right after each half's relus
                for t in range(NT):
                    nc.sync.dma_start(
                        out=out[p, :, :, t, :], in_=hn[:, :, t, :]
                    )
            else:
                nc.gpsimd.dma_start(out=out[p], in_=hn[:])
            h_prev = hn

    nc.compile()
    return nc


def _get_nc():
    if "nc" not in _CACHE:
        _CACHE["nc"] = _build_nc()
    return _CACHE["nc"]


def _pack_inputs(path_data, W, b):
    """Host-side packing into the DRAM layouts the kernel expects."""
    # lhsT for the two K=128 chunks: wh[kk, p, k, jj] = W[p, jj, 128k+kk]
    wh_np = np.ascontiguousarray(
        W[:, :, :H].reshape(P, H, 2, 128).transpose(3, 0, 2, 1)
    ).astype(BF16)
    # zero-padded K=64 lhsT blocks for the wp chunk:
    # wx[4p+r, p, m, j] = W[p, 128m+j, 256+r]
    wx_np = np.zeros((128, P, 2, 128), dtype=BF16)
    wxs = W[:, :, H:].reshape(P, 2, 128, PD).transpose(3, 0, 1, 2).astype(BF16)
    for p in range(P):
        wx_np[4 * p : 4 * p + 4, p] = wxs[:, p]
    # bias[j, p, m] = b[p, 128m+j]
    b_np = np.ascontiguousarray(b.reshape(P, 2, 128).transpose(2, 0, 1)).astype(
        np.float32
    )
    # per-core rhs for the wp chunk: pdx[4p+r, bb] = path_data[c*BS+bb, 4p+r]
    pdx_all = [
        np.ascontiguousarray(path_data[c * BS : (c + 1) * BS].T).astype(BF16)
        for c in range(NCORES)
    ]
    return wh_np, wx_np, b_np, pdx_all


def kernel(path_data, W, b):
    from concourse.bass_utils import run_bass_kernel_spmd

    path_data = np.asarray(path_data, dtype=np.float32)
    W = np.asarray(W, dtype=np.float32)
    b = np.asarray(b, dtype=np.float32)

    wh_np, wx_np, b_np, pdx_all = _pack_inputs(path_data, W, b)
    in_maps = [
        {"wh": wh_np, "wx": wx_np, "bias": b_np, "pdx": pdx_all[c]}
        for c in range(NCORES)
    ]

    nc = _get_nc()
    res = run_bass_kernel_spmd(nc, in_maps, core_ids=list(range(NCORES)))

    # out[p, jj, m, t, bb] -> full[c*BS + t*TN + bb, p*256 + m*128 + jj]
    full = np.concatenate(
        [
            np.asarray(r["out"])
            .transpose(3, 4, 0, 2, 1)
            .reshape(BS, P * H)
            .astype(np.float32)
            for r in res.results
        ],
        axis=0,
    )
    return full


# revision 52
# speedup vs baseline: 1.0725x; 1.0725x over previous
"""Trainium2 Bass kernel for CascadedPathEncoder.

Reference computation (per sample b):
    h_0 = relu(W_0 @ [0_256; wp_0] + b_0)
    h_p = relu(W_p @ [h_{p-1}; wp_p] + b_p)      p = 1..31
    out[b] = concat_p h_p                         -> [8192, 8192]

Strategy: pure data parallel over 8 NeuronCores (1024 batch rows each).
Per core the hidden state is kept transposed in SBUF as two bf16
[128, 1024] chunks (partition dim = hidden index). Each step runs, per
512-column batch tile and per output chunk m, three matmuls that
accumulate in PSUM: K=4 (the wp contribution, lhsT from W[:, :, 256:260])
and two K=128 chunks (lhsT from W[:, :, :256]). Bias + relu are fused on
the Activation engine (m=0) and Vector engine (m=1), writing the new
bf16 state chunk, which is also DMA'd to DRAM as the step's output
slice. Host un-transposes / re-assembles the full [8192, 8192] f32.
"""

import numpy as np
import ml_dtypes

BF16 = ml_dtypes.bfloat16
P = 32          # scan steps
PD = 4          # point dim
H = 256         # hidden dim
B = 8192        # global batch
NCORES = 8
BS = B // NCORES  # 1024 rows per core
TN = 512        # matmul moving free dim (one PSUM bank of f32; ISA max)
NT = BS // TN   # batch tiles per core

_CACHE = {}


def _build_nc():
    from contextlib import ExitStack

    import concourse.bass as bass
    import concourse.tile as tile
    from concourse import bacc, mybir

    dt = mybir.dt
    ts = bass.ts

    nc = bacc.Bacc(
        "TRN2", target_bir_lowering=False, debug=False, num_devices=NCORES
    )
    WH_CHUNK = 4  # steps per wh DMA chunk (pipeline weight loads)
    wh = nc.dram_tensor("wh", [128, P, 2, 256], dt.bfloat16, kind="ExternalInput").ap()
    # wx is stored as zero-padded K=64 lhsT blocks so both wx and pdx can be
    # read at matmul-legal base partitions (0/64) while their DMAs stay full
    # 128-partition width: wx[4p+r, p, m, j] = W[p, 128m+j, 256+r], zero
    # elsewhere; pdx[4p+r, b] = path_data[b, 4p+r].
    wx = nc.dram_tensor("wx", [128, P, 2, 128], dt.bfloat16, kind="ExternalInput").ap()
    pdx = nc.dram_tensor("pdx", [128, BS], dt.bfloat16, kind="ExternalInput").ap()
    bias = nc.dram_tensor("bias", [128, P, 2], dt.float32, kind="ExternalInput").ap()
    out = nc.dram_tensor(
        "out", [P, 128, 2, NT, TN], dt.bfloat16, kind="ExternalOutput"
    ).ap()

    with tile.TileContext(nc) as tc, ExitStack() as ctx:
        const = ctx.enter_context(tc.tile_pool(name="const", bufs=1))
        state = ctx.enter_context(tc.tile_pool(name="state", bufs=3))
        psum = ctx.enter_context(tc.tile_pool(name="psum", bufs=2, space="PSUM"))

        # Order the input DMAs by first-use; all transfers are full width.
        wx_sb = const.tile([128, P, 2, 128], dt.bfloat16)
        pdx_sb = const.tile([128, BS], dt.bfloat16)
        b_sb = const.tile([128, P, 2], dt.float32)
        wh_sb = const.tile([128, P, 2, 256], dt.bfloat16)

        nc.sync.dma_start(out=pdx_sb[:], in_=pdx[:])
        nc.sync.dma_start(out=wx_sb[:, 0:2, :, :], in_=wx[:, 0:2, :, :])
        nc.sync.dma_start(out=b_sb[:], in_=bias[:])
        # front-load the first half of wh: steps 1..15 are consumed faster
        # than the sync HWDGE ring can stream them, and stalls here are on
        # the critical path
        for g in range(2):
            sl = slice(g * WH_CHUNK, (g + 1) * WH_CHUNK)
            nc.sync.dma_start(out=wh_sb[:, sl, :, :], in_=wh[:, sl, :, :])
        nc.sync.dma_start(out=wx_sb[:, 2:8, :, :], in_=wx[:, 2:8, :, :])
        for g in range(2, P // WH_CHUNK):
            sl = slice(g * WH_CHUNK, (g + 1) * WH_CHUNK)
            nc.sync.dma_start(out=wh_sb[:, sl, :, :], in_=wh[:, sl, :, :])
            if g - 1 < 4:
                slx = slice(8 * (g - 1), 8 * g)
                nc.sync.dma_start(out=wx_sb[:, slx, :, :], in_=wx[:, slx, :, :])

        h_prev = None
        for p in range(P):
            ps = [
                psum.tile(
                    [128, NT, TN],
                    dt.float32,
                    tag=f"ps_m{m}",
                    name=f"ps_p{p}m{m}",
                )
                for m in range(2)
            ]
            # K=4 wp contribution opens each accumulation group. t is
            # innermost everywhere so consecutive matmuls share lhsT (one
            # weight load covers NT matmuls). k outer / m inner for the
            # K=128 chunks: the k=0 matmuls only need the previous step's
            # m=0 relu, giving the m=1 relu a longer window to complete.
            base = 64 * (p // 16)
            for m in range(2):
                for t in range(NT):
                    nc.tensor.matmul(
                        ps[m][:, t, :],
                        lhsT=wx_sb[base : base + 64, p, m, :],
                        rhs=pdx_sb[base : base + 64, ts(t, TN)],
                        start=True,
                        stop=(p == 0),
                    )
            if p > 0:
                for k in range(2):
                    for m in range(2):
                        for t in range(NT):
                            nc.tensor.matmul(
                                ps[m][:, t, :],
                                lhsT=wh_sb[:, p, k, ts(m, 128)],
                                rhs=h_prev[:, k, t, :],
                                start=False,
                                stop=(k == 1),
                            )
            hn = state.tile(
                [128, 2, NT, TN], dt.bfloat16, tag="h", name=f"h_p{p}"
            )
            for m in range(2):
                if m == 0:
                    nc.scalar.activation(
                        hn[:, m, :, :],
                        ps[m][:],
                        mybir.ActivationFunctionType.Relu,
                        bias=b_sb[:, p, m : m + 1],
                        scale=1.0,
                    )
                else:
                    nc.vector.tensor_scalar(
                        hn[:, m, :, :],
                        ps[m][:],
                        scalar1=b_sb[:, p, m : m + 1],
                        scalar2=0.0,
                        op0=mybir.AluOpType.add,
                        op1=mybir.AluOpType.max,
                    )
            if p >= P - 2:
                # tail: split by batch half so the final transfers start
                # right after each half's relus
                for t in range(NT):
                    nc.sync.dma_start(
                        out=out[p, :, :, t, :], in_=hn[:, :, t, :]
                    )
            else:
                nc.gpsimd.dma_start(out=out[p], in_=hn[:])
            h_prev = hn

    nc.compile()
    return nc


def _get_nc():
    if "nc" not in _CACHE:
        _CACHE["nc"] = _build_nc()
    return _CACHE["nc"]


def _pack_inputs(path_data, W, b):
    """Host-side packing into the DRAM layouts the kernel expects."""
    # lhsT for the two K=128 chunks: wh[kk, p, k, jj] = W[p, jj, 128k+kk]
    wh_np = np.ascontiguousarray(
        W[:, :, :H].reshape(P, H, 2, 128).transpose(3, 0, 2, 1)
    ).astype(BF16)
    # zero-padded K=64 lhsT blocks for the wp chunk:
    # wx[4p+r, p, m, j] = W[p, 128m+j, 256+r]
    wx_np = np.zeros((128, P, 2, 128), dtype=BF16)
    wxs = W[:, :, H:].reshape(P, 2, 128, PD).transpose(3, 0, 1, 2).astype(BF16)
    for p in range(P):
        wx_np[4 * p : 4 * p + 4, p] = wxs[:, p]
    # bias[j, p, m] = b[p, 128m+j]
    b_np = np.ascontiguousarray(b.reshape(P, 2, 128).transpose(2, 0, 1)).astype(
        np.float32
    )
    # per-core rhs for the wp chunk: pdx[4p+r, bb] = path_data[c*BS+bb, 4p+r]
    pdx_all = [
        np.ascontiguousarray(path_data[c * BS : (c + 1) * BS].T).astype(BF16)
        for c in range(NCORES)
    ]
    return wh_np, wx_np, b_np, pdx_all


def kernel(path_data, W, b):
    from concourse.bass_utils import run_bass_kernel_spmd

    path_data = np.asarray(path_data, dtype=np.float32)
    W = np.asarray(W, dtype=np.float32)
    b = np.asarray(b, dtype=np.float32)

    wh_np, wx_np, b_np, pdx_all = _pack_inputs(path_data, W, b)
    in_maps = [
        {"wh": wh_np, "wx": wx_np, "bias": b_np, "pdx": pdx_all[c]}
        for c in range(NCORES)
    ]

    nc = _get_nc()
    res = run_bass_kernel_spmd(nc, in_maps, core_ids=list(range(NCORES)))

    # out[p, jj, m, t, bb] -> full[c*BS + t*TN + bb, p*256 + m*128 + jj]
    full = np.concatenate(
        [
            np.asarray(r["out"])
            .transpose(3, 4, 0, 2, 1)
            .reshape(BS, P * H)
            .astype(np.float32)
            for r in res.results
        ],
        axis=0,
    )
    return full
